# revision 21
# baseline (speedup 1.0000x reference)
"""ChebNet (K=2, 2 layers) on 8 Trainium2 NeuronCores — v2.

Same algebra as v1:
    u    = dinv * (x @ W1)          # [N, 16] scaled projected features
    s[r] = sum_{e: row=r} u[col_e]  # graph op (GPSIMD ap_gather)
    tx1w = -dinv * s

v2 changes vs v1 (all aimed at the ap_gather request bound of ~107ns
per 4 indices):
  * groups are (src-pair, replica): Q7 cores 2p and 2p+1 both hold pair
    p's table and each serves a balanced half of every (dest, pair)
    edge set -> per-strip slot maxima drop ~35% (900k -> 583k padded
    slots per core per layer).
  * strips are single 128-dest pages; one fp32 p64 matmul per strip
    sums the 8 group partials straight into a node-major PSUM page.

Measured: 4.41 ms vs the 6.62 ms (dest-half, pair) baseline; both
layers' gathers run at the ap_gather request floor (~107 ns per
4-index read command, 98 chunks of 128*r idxs each).
"""

import numpy as np
import ml_dtypes

import concourse.bass as bass
import concourse.mybir as mybir
from concourse import library_config
from concourse.bass_utils import run_bass_kernel_spmd
from concourse.library_overlay import lower_extended_insts
from concourse.tile import TileContext

# ---- problem constants (hardcoded per contract) ----
N = 100000
F_IN = 500
FP = 512                      # F_IN padded to 4*128 contraction chunks
HID = 16
C = 7
N_CORES = 8
NL = N // N_CORES             # 12500 local rows per core
PAGES = 98                    # 98*128 = 12544 >= NL virtual rows
NV = PAGES * 128              # 12544
NSTRIPS = PAGES               # 98 strips of one page each
F2 = 8                        # layer-2 feature pad (7 -> 8)
KCH = 4                       # 512 = 4*128 contraction chunks
KC = FP // KCH
NPAIR = 4                     # src pairs
PCOLS = 2 * NL                # 25000 nodes per pair table
TCOLS = 25088                 # table cols (25000 + zero pad)
PADCOL = PCOLS                # index of first all-zero table column
RCAP = 32                     # max slots per gather chunk
TPAGES = 7                    # pages per phase-1 x tile
NTILES = PAGES // TPAGES      # 14

fp32 = mybir.dt.float32
fp16 = mybir.dt.float16
bf16 = mybir.dt.bfloat16
i16 = mybir.dt.int16

_cache = {}
_ctr = [0]


def _split_sync_waits(nc, max_waits=1):
    """walrus codegen accepts at most one sync wait per instruction; spill
    extras onto NOPs inserted just before, on the same engine."""
    for bass_bb in nc.bb_map.values():
        bb = bass_bb.bb
        new = []
        changed = False
        for inst in bb.instructions:
            si = inst.sync_info
            if si is not None and si.on_wait and len(si.on_wait) > max_waits:
                waits = list(si.on_wait)
                spill, keep = waits[:-max_waits], waits[-max_waits:]
                for i in range(0, len(spill), max_waits):
                    _ctr[0] += 1
                    nop = mybir.InstNoOp(
                        name=f"I-waitspill-{_ctr[0]}",
                        text_hint="waitspill",
                        bass_nofuse=True,
                    )
                    nop.engine = inst.engine
                    nop.sync_info = mybir.SyncInfo(
                        on_wait=spill[i : i + max_waits], on_update=[]
                    )
                    try:
                        nc.register_instruction(nop)
                    except Exception:
                        pass
                    new.append(nop)
                inst.sync_info = mybir.SyncInfo(
                    on_wait=keep, on_update=list(si.on_update or [])
                )
                changed = True
            new.append(inst)
        if changed:
            bb.instructions = new


def _load_tables(nc, table, full, fr):
    """16 casting DMAs: rank-major fp16 DRAM table -> fp32 SBUF pair
    tables. full is [8*fr, NL]; group g=2p+rep partitions 16g..16g+fr
    get rank 2p+cs at cols cs*NL."""
    for g in range(8):
        p = g // 2
        for cs in range(2):
            nc.gpsimd.dma_start(
                out=table[16 * g:16 * g + fr, cs * NL:(cs + 1) * NL],
                in_=full[fr * (2 * p + cs):fr * (2 * p + cs + 1), :])


def _build(chunks):
    """chunks: tuple of tuples; chunks[st] = slot-chunk sizes of strip st."""
    ni = int(sum(128 * r for ch in chunks for r in ch))
    maxr = max(r for ch in chunks for r in ch)
    nc = bass.Bass()

    xT = nc.declare_dram_parameter("xT", [FP, NV], bf16, isOutput=False)
    w1z = nc.declare_dram_parameter("w1z", [FP, HID], bf16, isOutput=False)
    w1u = nc.declare_dram_parameter("w1u", [FP, HID], bf16, isOutput=False)
    w2z = nc.declare_dram_parameter("w2z", [HID, F2], bf16, isOutput=False)
    w2u = nc.declare_dram_parameter("w2u", [HID, F2], bf16, isOutput=False)
    b1r = nc.declare_dram_parameter("b1r", [128, HID], fp32, isOutput=False)
    b2r = nc.declare_dram_parameter("b2r", [128, F2], fp32, isOutput=False)
    p64 = nc.declare_dram_parameter("p64", [128, HID], fp32, isOutput=False)
    dinvp = nc.declare_dram_parameter("dinvp", [128, PAGES], fp32,
                                      isOutput=False)
    dinvT = nc.declare_dram_parameter("dinvT", [HID, NV], fp16,
                                      isOutput=False)
    identp = nc.declare_dram_parameter("identp", [128, 128], bf16,
                                       isOutput=False)
    idx = nc.declare_dram_parameter("idx", [128, ni // 16], i16,
                                    isOutput=False)
    y = nc.declare_dram_parameter("y", [128, PAGES * F2], fp32,
                                  isOutput=True)

    u1T_loc = nc.dram_tensor("u1T_loc", [HID, NL], fp16)
    u2T_loc = nc.dram_tensor("u2T_loc", [F2, NL], fp16)
    u1T_full = nc.dram_tensor("u1T_full", [128, NL], fp16,
                              addr_space="Shared")
    u2T_full = nc.dram_tensor("u2T_full", [64, NL], fp16,
                              addr_space="Shared")
    groups = [list(range(N_CORES))]

    # flat gather-chunk list: (strip, chunk-slot-count, flat idx base)
    gch = []
    acc = 0
    for st in range(NSTRIPS):
        for k, r in enumerate(chunks[st]):
            gch.append((st, k, r, acc))
            acc += 128 * r

    with TileContext(nc) as tc:
        nc.gpsimd.load_library(library_config.ap_gather)
        with tc.tile_pool(name="persist", bufs=1) as pp:
            w1z_t = pp.tile([KC, KCH * HID], bf16)
            w1u_t = pp.tile([KC, KCH * HID], bf16)
            for k in range(KCH):
                nc.sync.dma_start(out=w1z_t[:, k * HID:(k + 1) * HID],
                                  in_=w1z[k * KC:(k + 1) * KC, :])
                nc.sync.dma_start(out=w1u_t[:, k * HID:(k + 1) * HID],
                                  in_=w1u[k * KC:(k + 1) * KC, :])
            w2z_t = pp.tile([HID, F2], bf16)
            nc.sync.dma_start(out=w2z_t[:], in_=w2z[:])
            w2u_t = pp.tile([HID, F2], bf16)
            nc.sync.dma_start(out=w2u_t[:], in_=w2u[:])
            b1_t = pp.tile([128, HID], fp32)
            nc.sync.dma_start(out=b1_t[:], in_=b1r[:])
            b2_t = pp.tile([128, F2], fp32)
            nc.sync.dma_start(out=b2_t[:], in_=b2r[:])
            p64_t = pp.tile([128, HID], fp32)
            nc.sync.dma_start(out=p64_t[:], in_=p64[:])
            dinvp_t = pp.tile([128, PAGES], fp32)
            nc.sync.dma_start(out=dinvp_t[:], in_=dinvp[:])
            ident = pp.tile([128, 128], bf16)
            nc.sync.dma_start(out=ident[:], in_=identp[:])
            idx_t = pp.tile([128, ni // 16], i16)
            nc.sync.dma_start(out=idx_t[:], in_=idx[:])

            z1 = pp.tile([128, PAGES * HID], fp16)
            z2 = pp.tile([128, PAGES * F2], bf16)
            lg = pp.tile([128, PAGES * F2], fp32)

            # ---- phase 1: z1 = x@W1_0 (node-major), u1T = dinv*(x@W1_1)
            # (feature-major), u1T -> DRAM -> AllGather ----
            dinvT_t = pp.tile([HID, NV], fp16)
            nc.sync.dma_start(out=dinvT_t[:], in_=dinvT[:])
            with tc.tile_pool(name="ph1", bufs=1) as p1, \
                 tc.tile_pool(name="xload", bufs=2) as xp, \
                 tc.tile_pool(name="ps1", bufs=2, space="PSUM") as ps1:
                stage = p1.tile([HID, NV], fp16)
                for t in range(NTILES):
                    c0 = t * TPAGES * 128            # node col base
                    xt = xp.tile([KC, KCH * TPAGES * 128], bf16, tag="xt")
                    nc.sync.dma_start(
                        out=xt[:].rearrange("p (k n) -> p k n", k=KCH),
                        in_=xT.rearrange("(k p) n -> p k n", k=KCH)
                        [:, :, c0:c0 + TPAGES * 128])
                    # u1T halves (448 cols each)
                    for hf in range(2):
                        pst = ps1.tile([HID, 448], fp32, tag="psT")
                        for k in range(KCH):
                            nc.tensor.matmul(
                                out=pst[:],
                                lhsT=w1u_t[:, k * HID:(k + 1) * HID],
                                rhs=xt[:, k * TPAGES * 128 + hf * 448:
                                       k * TPAGES * 128 + hf * 448 + 448],
                                start=(k == 0), stop=(k == KCH - 1))
                        nc.vector.tensor_tensor(
                            out=stage[:, c0 + hf * 448:c0 + hf * 448 + 448],
                            in0=pst[:],
                            in1=dinvT_t[:, c0 + hf * 448:c0 + hf * 448 + 448],
                            op=mybir.AluOpType.mult)
                    # z1 pages
                    for pg_ in range(TPAGES):
                        pg = t * TPAGES + pg_
                        psz = ps1.tile([128, HID], fp32, tag="psZ")
                        for k in range(KCH):
                            nc.tensor.matmul(
                                out=psz[:],
                                lhsT=xt[:, k * TPAGES * 128 + pg_ * 128:
                                        k * TPAGES * 128 + pg_ * 128 + 128],
                                rhs=w1z_t[:, k * HID:(k + 1) * HID],
                                start=(k == 0), stop=(k == KCH - 1))
                        nc.vector.tensor_copy(
                            out=z1[:, pg * HID:(pg + 1) * HID], in_=psz[:])
                nc.sync.dma_start(out=u1T_loc[:, :], in_=stage[:, :NL])
            nc.gpsimd.collective_compute(
                "AllGather", mybir.AluOpType.bypass,
                replica_groups=groups,
                ins=[u1T_loc[:, :].opt()],
                outs=[u1T_full[:, :].opt()],
            )

            # ---- gather phase ----
            with tc.tile_pool(name="gat", bufs=1) as gp, \
                 tc.tile_pool(name="msgs", bufs=2) as mp, \
                 tc.tile_pool(name="sfap", bufs=2) as sp, \
                 tc.tile_pool(name="hpg", bufs=3) as hp, \
                 tc.tile_pool(name="ps2", bufs=2, space="PSUM") as ps2:
                table = gp.tile([128, TCOLS], fp32)
                nc.vector.memset(table[:, PCOLS:TCOLS], 0.0)
                tmp = gp.tile([128, 128], fp32)
                tmpc = gp.tile([128, HID], fp32)
                _load_tables(nc, table, u1T_full, HID)

                def gather_layer(dest_d, scale_bias_relu):
                    """gather+reduce into sfa tile, then one p64 matmul
                    per strip -> node-major psum -> combine via
                    scale_bias_relu(pg, psum_ap)."""
                    sfa_t = None
                    for (st, k, r, base) in gch:
                        m = mp.tile([128, 128 * maxr], fp32, tag="m")
                        nc.gpsimd.ap_gather(
                            out_ap=m[:, :128 * r],
                            in_ap=table[:],
                            idxs_ap=idx_t[:, base // 16:(base + 128 * r) // 16],
                            channels=128, num_elems=TCOLS, d=1,
                            num_idxs=128 * r)
                        if k == 0:
                            sfa_t = sp.tile([128, 128], fp32, tag="sfa")
                        dst = sfa_t[:] if k == 0 else tmp[:]
                        nc.vector.tensor_reduce(
                            out=dst.unsqueeze(-1),
                            in_=m[:, :128 * r].rearrange(
                                "p (c r) -> p c r", r=r),
                            axis=mybir.AxisListType.X,
                            op=mybir.AluOpType.add)
                        if k > 0:
                            nc.vector.tensor_tensor(
                                out=sfa_t[:], in0=sfa_t[:],
                                in1=tmp[:], op=mybir.AluOpType.add)
                        if k == len(chunks[st]) - 1:
                            pss = ps2.tile([128, dest_d], fp32, tag="psS")
                            nc.tensor.matmul(
                                out=pss[:],
                                lhsT=sfa_t[:],
                                rhs=p64_t[:, :dest_d],
                                start=True, stop=True)
                            scale_bias_relu(st, pss)

                def combine1(pg, pss):
                    # h = relu(z1 - dinv*s + b1); transpose; z2/u2T mms
                    hpg_t = hp.tile([128, HID], bf16, tag="h")
                    nc.vector.tensor_tensor(
                        out=tmpc[:], in0=pss[:],
                        in1=dinvp_t[:, pg:pg + 1].to_broadcast([128, HID]),
                        op=mybir.AluOpType.mult)
                    nc.vector.tensor_tensor(
                        out=tmpc[:], in0=z1[:, pg * HID:(pg + 1) * HID],
                        in1=tmpc[:], op=mybir.AluOpType.subtract)
                    nc.vector.tensor_tensor(
                        out=tmpc[:], in0=tmpc[:], in1=b1_t[:],
                        op=mybir.AluOpType.add)
                    nc.vector.tensor_scalar(
                        out=hpg_t[:], in0=tmpc[:], scalar1=0.0,
                        scalar2=None, op0=mybir.AluOpType.max)
                    # transpose h page -> [HID, 128]
                    pst = ps2.tile([HID, 128], bf16, tag="psT2")
                    nc.tensor.transpose(out=pst[:], in_=hpg_t[:],
                                        identity=ident[:])
                    hT = hp.tile([HID, 128], bf16, tag="hT")
                    nc.vector.tensor_copy(out=hT[:], in_=pst[:])
                    psz2 = ps2.tile([128, F2], fp32, tag="psZ2")
                    nc.tensor.matmul(out=psz2[:], lhsT=hT[:], rhs=w2z_t[:],
                                     start=True, stop=True)
                    nc.vector.tensor_copy(
                        out=z2[:, pg * F2:(pg + 1) * F2], in_=psz2[:])
                    psu2 = ps2.tile([F2, 128], fp32, tag="psU2")
                    nc.tensor.matmul(out=psu2[:], lhsT=w2u_t[:], rhs=hT[:],
                                     start=True, stop=True)
                    u2s = hp.tile([F2, 128], fp16, tag="u2s")
                    nc.vector.tensor_tensor(
                        out=u2s[:], in0=psu2[:],
                        in1=dinvT_t[:F2, pg * 128:(pg + 1) * 128],
                        op=mybir.AluOpType.mult)
                    hi = min((pg + 1) * 128, NL)
                    if hi > pg * 128:
                        nc.sync.dma_start(
                            out=u2T_loc[:, pg * 128:hi],
                            in_=u2s[:, :hi - pg * 128])

                gather_layer(HID, combine1)

                nc.gpsimd.collective_compute(
                    "AllGather", mybir.AluOpType.bypass,
                    replica_groups=groups,
                    ins=[u2T_loc[:, :].opt()],
                    outs=[u2T_full[:, :].opt()],
                )
                _load_tables(nc, table, u2T_full, F2)

                def combine2(pg, pss):
                    # lg = z2 - dinv*s2 + b2
                    nc.vector.tensor_tensor(
                        out=tmpc[:, :F2], in0=pss[:],
                        in1=dinvp_t[:, pg:pg + 1].to_broadcast([128, F2]),
                        op=mybir.AluOpType.mult)
                    nc.vector.tensor_tensor(
                        out=tmpc[:, :F2], in0=z2[:, pg * F2:(pg + 1) * F2],
                        in1=tmpc[:, :F2], op=mybir.AluOpType.subtract)
                    nc.vector.tensor_tensor(
                        out=lg[:, pg * F2:(pg + 1) * F2],
                        in0=tmpc[:, :F2], in1=b2_t[:],
                        op=mybir.AluOpType.add)

                gather_layer(F2, combine2)

            # ---- log_softmax (node-major slab) + output ----
            with tc.tile_pool(name="fin", bufs=1) as wp:
                lgv = lg[:].rearrange("p (a f) -> p a f", f=F2)
                mxr = wp.tile([128, PAGES], fp32)
                nc.vector.tensor_reduce(
                    out=mxr[:].unsqueeze(-1),
                    in_=lgv[:, :, :C],
                    axis=mybir.AxisListType.X, op=mybir.AluOpType.max)
                d0 = wp.tile([128, PAGES * F2], fp32)
                nc.vector.tensor_tensor(
                    out=d0[:].rearrange("p (a f) -> p a f", f=F2),
                    in0=lgv,
                    in1=mxr[:].unsqueeze(-1).to_broadcast([128, PAGES, F2]),
                    op=mybir.AluOpType.subtract)
                ex = wp.tile([128, PAGES * F2], fp32)
                nc.scalar.activation(out=ex[:], in_=d0[:],
                                     func=mybir.ActivationFunctionType.Exp)
                sm = wp.tile([128, PAGES], fp32)
                nc.vector.tensor_reduce(
                    out=sm[:].unsqueeze(-1),
                    in_=ex[:].rearrange("p (a f) -> p a f", f=F2)[:, :, :C],
                    axis=mybir.AxisListType.X, op=mybir.AluOpType.add)
                nc.scalar.activation(out=sm[:], in_=sm[:],
                                     func=mybir.ActivationFunctionType.Ln)
                res = wp.tile([128, PAGES * F2], fp32)
                nc.vector.tensor_tensor(
                    out=res[:].rearrange("p (a f) -> p a f", f=F2),
                    in0=d0[:].rearrange("p (a f) -> p a f", f=F2),
                    in1=sm[:].unsqueeze(-1).to_broadcast([128, PAGES, F2]),
                    op=mybir.AluOpType.subtract)
                nc.sync.dma_start(out=y[:], in_=res[:])
    lower_extended_insts(nc)
    _split_sync_waits(nc)
    return nc


# --------------------------------------------------------------------------
# host-side sharding / layout prep (pure data layout)
# --------------------------------------------------------------------------
def _prep(x, edge_index, W1_0, W1_1, b1, W2_0, W2_1, b2):
    x = np.asarray(x, np.float32)
    ei = np.asarray(edge_index)
    row, col = ei[0].astype(np.int64), ei[1].astype(np.int64)

    order_e = np.argsort(row, kind="stable")
    row_s, col_s = row[order_e], col[order_e]
    deg_full = np.bincount(row_s, minlength=N).astype(np.int64)
    row_ptr = np.zeros(N + 1, np.int64)
    np.cumsum(deg_full, out=row_ptr[1:])

    # per-core relabeling: sort dests by max per-(pair,replica) count desc
    orders = []
    newpos = np.empty(N, np.int64)
    for cidx in range(N_CORES):
        lo = cidx * NL
        e0, e1 = row_ptr[lo], row_ptr[lo + NL]
        v0 = row_s[e0:e1] - lo
        p0 = col_s[e0:e1] // PCOLS
        cp = np.bincount(v0 * NPAIR + p0,
                         minlength=NL * NPAIR).reshape(NL, NPAIR)
        keym = (cp + 1) // 2
        o = np.argsort(-keym.max(axis=1), kind="stable")
        orders.append(o)
        newpos[lo + o] = lo + np.arange(NL)

    dinv_full = np.where(
        deg_full > 0, 1.0 / np.sqrt(np.maximum(deg_full, 1.0)),
        0.0).astype(np.float32)

    # pass 1: per-core edge -> (group, strip, col, slot); strip slot maxima
    per_edge = []
    cnt_all = []
    for cidx in range(N_CORES):
        e0, e1 = row_ptr[cidx * NL], row_ptr[(cidx + 1) * NL]
        ne = e1 - e0
        vr = newpos[row_s[e0:e1]] - cidx * NL
        sp = newpos[col_s[e0:e1]]
        pair = sp // PCOLS
        paircol = (sp % PCOLS).astype(np.int64)
        # split each (dest, pair) set into two balanced replica streams
        keyp = vr * NPAIR + pair
        cntp = np.bincount(keyp, minlength=NV * NPAIR)
        ksp = np.argsort(keyp, kind="stable")
        startsp = np.cumsum(cntp) - cntp
        slotp = np.empty(ne, np.int64)
        slotp[ksp] = np.arange(ne) - startsp[keyp[ksp]]
        ceilp = (cntp + 1) // 2
        rep = (slotp >= ceilp[keyp]).astype(np.int64)
        g = 2 * pair + rep
        slot = np.where(rep == 1, slotp - ceilp[keyp], slotp)
        key = g * NV + vr
        cnt = np.bincount(key, minlength=8 * NV)
        per_edge.append((g, vr, paircol, slot))
        cnt_all.append(cnt)
    rst = np.zeros(NSTRIPS, np.int64)
    for cnt in cnt_all:
        m = cnt.reshape(8, NSTRIPS, 128).max(axis=(0, 2))
        rst = np.maximum(rst, m)
    chunks = tuple(
        tuple([RCAP] * int(r // RCAP) + ([int(r % RCAP)] if r % RCAP else [])
              ) if r > 0 else (1,)
        for r in rst)

    # flat base per (strip, chunk)
    base = {}
    acc = 0
    for st in range(NSTRIPS):
        for k, r in enumerate(chunks[st]):
            base[(st, k)] = acc
            acc += 128 * r
    ni = acc
    assert ni % 16 == 0

    w1z = np.zeros((FP, HID), np.float32)
    w1z[:F_IN] = np.asarray(W1_0, np.float32)
    w1z = w1z.astype(ml_dtypes.bfloat16)
    w1u = np.zeros((FP, HID), np.float32)
    w1u[:F_IN] = np.asarray(W1_1, np.float32)
    w1u = w1u.astype(ml_dtypes.bfloat16)
    w2zp = np.zeros((HID, F2), np.float32)
    w2zp[:, :C] = np.asarray(W2_0, np.float32)
    w2up = np.zeros((HID, F2), np.float32)
    w2up[:, :C] = np.asarray(W2_1, np.float32)
    b1rep = np.tile(np.asarray(b1, np.float32)[None, :], (128, 1))
    b2rep = np.zeros((128, F2), np.float32)
    b2rep[:, :C] = np.asarray(b2, np.float32)[None, :]
    p64a = np.zeros((128, HID), np.float32)
    for p in range(8):
        p64a[16 * p:16 * p + 16] = np.eye(HID)
    identa = np.eye(128, dtype=np.float32).astype(ml_dtypes.bfloat16)

    in_maps = []
    for cidx in range(N_CORES):
        lo = cidx * NL
        o = orders[cidx]
        g, vr, paircol, slot = per_edge[cidx]

        # flat position of each edge in its group's index stream
        st_e = vr // 128
        c_e = vr % 128
        k_e = slot // RCAP
        sk_e = slot % RCAP
        maxk = max(len(ch) for ch in chunks)
        cs_tab = np.ones((NSTRIPS, maxk), np.int64)
        b_tab = np.zeros((NSTRIPS, maxk), np.int64)
        for s in range(NSTRIPS):
            for k, r in enumerate(chunks[s]):
                cs_tab[s, k] = r
                b_tab[s, k] = base[(s, k)]
        flat = b_tab[st_e, k_e] + c_e * cs_tab[st_e, k_e] + sk_e
        idx_arr = np.full((8, ni), PADCOL, np.int16)
        idx_arr[g, flat] = paircol.astype(np.int16)
        idx16 = idx_arr.reshape(8, ni // 16, 16).transpose(0, 2, 1) \
            .reshape(128, ni // 16).copy()

        xTc = np.zeros((FP, NV), np.float32)
        xTc[:F_IN, :NL] = x[lo:lo + NL][o].T
        xTc = xTc.astype(ml_dtypes.bfloat16)

        dv = np.zeros(NV, np.float32)
        dv[:NL] = dinv_full[lo:lo + NL][o]
        dinvp_a = dv.reshape(PAGES, 128).T.copy()
        dinvT_a = np.tile(dv[None, :], (HID, 1)).astype(np.float16)

        in_maps.append(dict(
            xT=xTc, w1z=w1z, w1u=w1u,
            w2z=w2zp.astype(ml_dtypes.bfloat16),
            w2u=w2up.astype(ml_dtypes.bfloat16),
            b1r=b1rep, b2r=b2rep, p64=p64a,
            dinvp=dinvp_a, dinvT=dinvT_a, identp=identa,
            idx=idx16,
        ))
    return in_maps, chunks, orders


def kernel(x, edge_index, W1_0, W1_1, b1, W2_0, W2_1, b2):
    in_maps, chunks, orders = _prep(x, edge_index, W1_0, W1_1, b1,
                                    W2_0, W2_1, b2)
    if chunks not in _cache:
        _cache[chunks] = _build(chunks)
    nc = _cache[chunks]
    res = run_bass_kernel_spmd(nc, in_maps, list(range(N_CORES)))
    out = np.empty((N, C), np.float32)
    for i in range(N_CORES):
        yv = res.results[i]["y"].reshape(128, PAGES, F2)[:, :, :C]
        out[i * NL + orders[i]] = yv.transpose(1, 0, 2).reshape(NV, C)[:NL]
    return out
